# revision 39
# baseline (speedup 1.0000x reference)
"""Trainium2 Bass kernel for nn_BatchEncoder (gnn_message_passing).

Reference computation (shapes hardcoded from the problem spec):
    nodes [1M, 128] f32, W1 [8,256,256], b1 [8,256], W2 [8,256,128], b2 [8,128]
    idx [8, 65536, 2] i64, out_idx [8, 65536] i64
    x   = nodes[idx].reshape(8, 65536, 256)
    h   = relu(x @ W1 + b1)
    out = h @ W2 + b2                       # [8, 65536, 128]
    new_nodes = nodes.at[out_idx.ravel()].set(out.reshape(-1, 128))

Sharding: data-parallel over the Mt (items) axis across 8 NeuronCores; each
core computes 8192 items of each of the 8 types.  The per-core nodes table is
the per-type deduplicated set of rows referenced by that core's indices
(dedup keeps indices within int16 range for the SWDGE gather), converted to
bf16 on the host and padded to a fixed 16384 rows/type so all cores share one
SPMD program.

Per-core device dataflow (engines pipelined by the Tile framework):
  dma_gather(transpose=True)  -> xT [128e, m] bf16 tiles directly in SBUF
                                 (no PE transposes needed; 16-bit-granularity
                                 xbar transpose during the gather itself)
  GEMM1 (bf16)                -> hT [f, m] in PSUM (2x2 K/f blocking)
  ACT relu+b1                 -> SBUF bf16
  GEMM2 (bf16)                -> outT [g, m] in PSUM
  DVE +b2                     -> SBUF bf16
  DMA store outT              -> DRAM [128, items] (host undoes the transpose)
"""

import os

import numpy as np
from ml_dtypes import bfloat16

# ---- problem constants (from spec) ----
N_NODES = 1_000_000
E = 128            # embedding dim
T = 8              # types
MT = 65536         # items per type
N_CORES = 8

# ---- sharding / tiling parameters ----
P = 128                                # partitions
M_PER_CORE = MT // N_CORES             # 8192 items per (type, core)
TT_ROWS = M_PER_CORE * 2               # 16384: max unique rows per (core, type)
CHUNK = 512                            # items per dma_gather (transpose-mode limit)
TILE_M = 512                           # items per GEMM tile (PSUM bank = 512 f32)


def _build_program(n_types=T, m_per_core=M_PER_CORE, tt_rows=TT_ROWS,
                   chunk=CHUNK, tile_m=TILE_M, num_devices=N_CORES,
                   gather_queues=4, gather_sp=True, dma_scratch=16384,
                   tgather=True, xbufs=4, group=8, variant="dmat"):
    """Build + compile the per-core Bass program. Returns the Bacc instance.

    variant="dmat": host materializes the gathered activations (the
                   sharding the problem hint suggests); the device streams
                   them with HWDGE dma_start_transpose (xbar) straight into
                   xT [e, m] SBUF tiles.  No SWDGE descriptors at all.
    variant="tg" : dma_gather(transpose=True) delivers xT directly
                   (chunk <= 512 hardware limit).
    variant="plain": plain dma_gather ([m, e] rows) + PE transposes
                   (bf16 identity matmuls), chunk <= 1024.
    """
    from contextlib import ExitStack

    import concourse.tile as tile
    from concourse import bacc, mybir
    from concourse.masks import make_identity

    f32 = mybir.dt.float32
    bf16 = mybir.dt.bfloat16
    i16 = mybir.dt.int16

    if variant == "tg":
        tgather = True
    elif variant == "plain":
        tgather = False

    n_chunks = m_per_core // chunk
    tiles_per_chunk = chunk // tile_m
    n_tiles = m_per_core // tile_m
    idx_cols = chunk // 16                     # int16 idx columns per (t, ch, op)

    nc = bacc.Bacc("TRN2", target_bir_lowering=False, debug=False,
                   num_devices=num_devices, num_swdge_queues=gather_queues,
                   dynamic_dma_scratch_size=dma_scratch)

    if variant == "dmat":
        xin_t = nc.dram_tensor("xin", [n_types * m_per_core * 2, E], bf16,
                               kind="ExternalInput")
        xin_d = xin_t.ap()
    else:
        nodes_t = nc.dram_tensor("nodes", [n_types * tt_rows, E], bf16,
                                 kind="ExternalInput")
        idx_t = nc.dram_tensor("idx",
                               [P, n_types * n_chunks * 2 * idx_cols], i16,
                               kind="ExternalInput")
        nodes = nodes_t.ap()
        idx_d = idx_t.ap()
    w1_t = nc.dram_tensor("w1", [P, n_types * 2 * 2 * E], bf16, kind="ExternalInput")
    w2_t = nc.dram_tensor("w2", [P, n_types * 2 * E], bf16, kind="ExternalInput")
    b1_t = nc.dram_tensor("b1", [P, n_types * 2], f32, kind="ExternalInput")
    b2_t = nc.dram_tensor("b2", [P, n_types], f32, kind="ExternalInput")
    out_t = nc.dram_tensor("out", [P, n_types * m_per_core], bf16,
                           kind="ExternalOutput")

    w1_d, w2_d, b1_d, b2_d = w1_t.ap(), w2_t.ap(), b1_t.ap(), b2_t.ap()
    out_d = out_t.ap()

    with tile.TileContext(nc) as tc, ExitStack() as ctx:
        nc = tc.nc
        const = ctx.enter_context(tc.tile_pool(name="const", bufs=1))

        if variant == "plain":
            ident_f32 = const.tile([P, P], f32)
            make_identity(nc, ident_f32[:])
            ident = const.tile([P, P], bf16)
            nc.vector.tensor_copy(out=ident[:], in_=ident_f32[:])

        w1_sb = const.tile([P, n_types * 2 * 2 * E], bf16)
        nc.sync.dma_start(out=w1_sb[:], in_=w1_d[:])
        w2_sb = const.tile([P, n_types * 2 * E], bf16)
        nc.sync.dma_start(out=w2_sb[:], in_=w2_d[:])
        b1_sb = const.tile([P, n_types * 2], f32)
        nc.sync.dma_start(out=b1_sb[:], in_=b1_d[:])
        b2_sb = const.tile([P, n_types], f32)
        nc.sync.dma_start(out=b2_sb[:], in_=b2_d[:])
        if variant != "dmat":
            idx_sb = const.tile([P, n_types * n_chunks * 2 * idx_cols], i16)
            nc.sync.dma_start(out=idx_sb[:], in_=idx_d[:])

        xpool = ctx.enter_context(tc.tile_pool(name="x", bufs=xbufs))
        htp = ctx.enter_context(tc.tile_pool(name="htp", bufs=2, space="PSUM"))
        hts = ctx.enter_context(tc.tile_pool(name="hts", bufs=3))
        pop = ctx.enter_context(tc.tile_pool(name="pop", bufs=3, space="PSUM"))
        osb = ctx.enter_context(tc.tile_pool(name="osb", bufs=3))
        if variant == "plain":
            xtp = ctx.enter_context(tc.tile_pool(name="xtp", bufs=2, space="PSUM"))
            xts = ctx.enter_context(tc.tile_pool(name="xts", bufs=2))
        k_blk = tile_m // P

        def stage1(t, rhs_op):
            """GEMM1 + relu for one tile; returns the bf16 hT halves.

            h tiles are split per f-half so relu(fh=0) overlaps GEMM1(fh=1)."""
            ht_ps0 = htp.tile([P, tile_m], f32, tag="h0")
            ht_ps1 = htp.tile([P, tile_m], f32, tag="h1")
            ht_sb0 = hts.tile([P, tile_m], bf16, tag="hs0")
            ht_sb1 = hts.tile([P, tile_m], bf16, tag="hs1")
            ht_ps = [ht_ps0, ht_ps1]
            ht_sb = [ht_sb0, ht_sb1]
            for fh in range(2):
                for eh in range(2):
                    lhsT = w1_sb[:, ((t * 2 + eh) * 2 + fh) * E:
                                 ((t * 2 + eh) * 2 + fh + 1) * E]
                    nc.tensor.matmul(
                        out=ht_ps[fh][:],
                        lhsT=lhsT, rhs=rhs_op[eh],
                        start=(eh == 0), stop=(eh == 1))
                nc.scalar.activation(
                    out=ht_sb[fh][:],
                    in_=ht_ps[fh][:],
                    func=mybir.ActivationFunctionType.Relu,
                    bias=b1_sb[:, t * 2 + fh:t * 2 + fh + 1])
            return ht_sb

        def stage2(t, ht_sb, o_out):
            """GEMM2 + bias into o_out ([P, tile_m] slice)."""
            o_ps = pop.tile([P, tile_m], f32)
            for fh in range(2):
                lhsT = w2_sb[:, (t * 2 + fh) * E:(t * 2 + fh + 1) * E]
                nc.tensor.matmul(out=o_ps[:], lhsT=lhsT, rhs=ht_sb[fh][:],
                                 start=(fh == 0), stop=(fh == 1))
            nc.vector.tensor_add(
                o_out, o_ps[:],
                b2_sb[:, t:t + 1].to_broadcast([P, tile_m]))

        def compute_tile(t, rhs_op, o_out):
            stage2(t, stage1(t, rhs_op), o_out)

        if variant == "dmat":
            # software-pipelined emission: stage2 (GEMM2) of tile N-1 is
            # emitted after stage1 (GEMM1+relu) of tile N so the PE queue
            # never head-blocks on the relu latency.
            prev = None   # (t, ht_sb, og_slice, store_or_None)
            for t in range(n_types):
                for g in range(n_tiles // group):
                    # one xbar-transposed load feeds `group` compute tiles
                    xt_ = xpool.tile([P, group * 2 * tile_m], bf16, tag="x")
                    base = (t * n_tiles + g * group) * 2 * tile_m
                    nc.sync.dma_start_transpose(
                        out=xt_[:],
                        in_=xin_d[base:base + group * 2 * tile_m, :])
                    og = osb.tile([P, group * tile_m], bf16, tag="o")
                    ob = t * m_per_core + g * group * tile_m
                    for k in range(group):
                        rhs_op = [xt_[:, ((k * 2 + eh) * tile_m):
                                      ((k * 2 + eh + 1) * tile_m)]
                                  for eh in range(2)]
                        ht_sb = stage1(t, rhs_op)
                        if prev is not None:
                            pt, phs, pslice, pstore = prev
                            stage2(pt, phs, pslice)
                            if pstore is not None:
                                nc.scalar.dma_start(out=pstore[0],
                                                    in_=pstore[1])
                        store = ((out_d[:, ob:ob + group * tile_m], og[:])
                                 if k == group - 1 else None)
                        prev = (t, ht_sb, og[:, k * tile_m:(k + 1) * tile_m],
                                store)
            if prev is not None:
                pt, phs, pslice, pstore = prev
                stage2(pt, phs, pslice)
                if pstore is not None:
                    nc.scalar.dma_start(out=pstore[0], in_=pstore[1])
        else:
            for t in range(n_types):
              for ch in range(n_chunks):
                # ---- gather: xT[e, m] directly, or [m, e] rows + PE transpose
                x_op = []
                for op in range(2):
                    xt_ = xpool.tile([P, chunk], bf16, tag=f"x{op}")
                    col = ((t * n_chunks + ch) * 2 + op) * idx_cols
                    if tgather:
                        out_ap = xt_[:].rearrange("p (o m) -> p o m", o=1)
                    else:
                        out_ap = xt_[:].rearrange("p (k g) -> p k g", g=E)
                    nc.gpsimd.dma_gather(
                        out_ap=out_ap,
                        in_ap=nodes[t * tt_rows:(t + 1) * tt_rows, :],
                        idxs_ap=idx_sb[:, col:col + idx_cols],
                        num_idxs=chunk,
                        num_idxs_reg=chunk,
                        elem_size=E,
                        transpose=tgather,
                        single_packet=gather_sp,
                        queue_num=((t * n_chunks + ch) * 2 + op) % gather_queues,
                    )
                    x_op.append(xt_)

                for ti in range(tiles_per_chunk):
                    if tgather:
                        rhs_op = [x_op[eh][:, ti * tile_m:(ti + 1) * tile_m]
                                  for eh in range(2)]
                    else:
                        # PE-transpose [m, e] blocks into xT [e, m]
                        xt_ps = xtp.tile([P, 2 * tile_m], bf16)
                        for op in range(2):
                            for kk in range(k_blk):
                                src = x_op[op][:, (ti * k_blk + kk) * P:
                                               (ti * k_blk + kk + 1) * P]
                                nc.tensor.transpose(
                                    out=xt_ps[:, op * tile_m + kk * P:
                                              op * tile_m + (kk + 1) * P],
                                    in_=src, identity=ident[:])
                        xt_sb = xts.tile([P, 2 * tile_m], bf16)
                        nc.any.tensor_copy(out=xt_sb[:, :tile_m],
                                           in_=xt_ps[:, :tile_m])
                        nc.any.tensor_copy(out=xt_sb[:, tile_m:],
                                           in_=xt_ps[:, tile_m:])
                        rhs_op = [xt_sb[:, eh * tile_m:(eh + 1) * tile_m]
                                  for eh in range(2)]

                    o_sb = osb.tile([P, tile_m], bf16, tag="o")
                    compute_tile(t, rhs_op, o_sb[:])
                    ob = t * m_per_core + (ch * tiles_per_chunk + ti) * tile_m
                    nc.scalar.dma_start(out=out_d[:, ob:ob + tile_m],
                                        in_=o_sb[:])

    nc.compile()
    return nc


_PROG_CACHE = {}


def _get_program(**kw):
    key = tuple(sorted(kw.items()))
    if key not in _PROG_CACHE:
        _PROG_CACHE[key] = _build_program(**kw)
    return _PROG_CACHE[key]


def _prep_core_inputs(nodes, w1b, b1f, w2b, b2f, idx, core,
                      n_types=T, m_per_core=M_PER_CORE, tt_rows=TT_ROWS,
                      chunk=CHUNK):
    """Host-side shard prep for one core: per-type dedup of node rows (bf16),
    int16 remapped indices in the dma_gather 16-partition-wrap layout."""
    n_chunks = m_per_core // chunk

    sl = idx[:, core * m_per_core:(core + 1) * m_per_core, :]  # [T, m, 2]
    nodes_sub = np.zeros((n_types * tt_rows, E), dtype=bfloat16)
    remap = np.zeros((n_types, m_per_core, 2), dtype=np.int16)
    for t in range(n_types):
        uniq, inv = np.unique(sl[t].ravel(), return_inverse=True)
        nodes_sub[t * tt_rows:t * tt_rows + len(uniq)] = nodes[uniq]
        remap[t] = inv.astype(np.int16).reshape(m_per_core, 2)

    # idx16[p, t, ch, op, s] = remap[t, ch*chunk + s*16 + (p%16), op]
    r = remap.reshape(n_types, n_chunks, chunk // 16, 16, 2)   # [t,ch,s,w,op]
    idx_dev = np.ascontiguousarray(
        np.tile(r.transpose(3, 0, 1, 4, 2), (8, 1, 1, 1, 1))).reshape(P, -1)

    return {
        "nodes": nodes_sub,
        "idx": idx_dev,
        "w1": w1b,
        "w2": w2b,
        "b1": b1f,
        "b2": b2f,
    }


def _prep_core_inputs_dmat(nodes, w1b, b1f, w2b, b2f, idx, core,
                           n_types=T, m_per_core=M_PER_CORE, tile_m=TILE_M):
    """Host-side shard prep, dmat variant: materialize this core's slice of
    the gathered activations (bf16), rows grouped [t, tile, op, j] so the
    device can stream them with transposed DMA loads."""
    n_tiles = m_per_core // tile_m
    sl = idx[:, core * m_per_core:(core + 1) * m_per_core, :]   # [T, m, 2]
    x = nodes[sl]                                               # [T, m, 2, E]
    xr = x.reshape(n_types, n_tiles, tile_m, 2, E).transpose(0, 1, 3, 2, 4)
    xin = np.ascontiguousarray(xr).reshape(-1, E).astype(bfloat16)
    return {
        "xin": xin,
        "w1": w1b,
        "w2": w2b,
        "b1": b1f,
        "b2": b2f,
    }


_LAST_RESULTS = {}


def kernel(nodes, W1, b1, W2, b2, idx, out_idx):
    from concourse.bass_utils import run_bass_kernel_spmd

    nodes = np.asarray(nodes, dtype=np.float32)
    W1 = np.asarray(W1, dtype=np.float32)
    b1 = np.asarray(b1, dtype=np.float32)
    W2 = np.asarray(W2, dtype=np.float32)
    b2 = np.asarray(b2, dtype=np.float32)
    idx = np.asarray(idx)
    out_idx_np = np.asarray(out_idx)

    cfg = {}
    for key, cast in (("chunk", int), ("gather_queues", int),
                      ("dma_scratch", int), ("variant", str),
                      ("xbufs", int)):
        v = os.environ.get("K_" + key.upper())
        if v is not None:
            cfg[key] = cast(v)
    nc = _get_program(**cfg)
    variant = cfg.get("variant", "dmat")

    # weight relayouts shared by all cores (bf16 for the PE):
    # w1_dev[p, t, eh, fh, fi] = W1[t, eh*128+p, fh*128+fi]
    w1r = W1.reshape(T, 2, P, 2, E)
    w1b = np.ascontiguousarray(
        w1r.transpose(2, 0, 1, 3, 4)).reshape(P, -1).astype(bfloat16)
    # w2_dev[p, t, fh, g] = W2[t, fh*128+p, g]
    w2r = W2.reshape(T, 2, P, E)
    w2b = np.ascontiguousarray(
        w2r.transpose(2, 0, 1, 3)).reshape(P, -1).astype(bfloat16)
    # b1_dev[p, t*2+fh] = b1[t, fh*128+p]
    b1r = b1.reshape(T, 2, P)
    b1f = np.ascontiguousarray(b1r.transpose(2, 0, 1)).reshape(P, -1)
    b1f = b1f.astype(np.float32)
    # b2_dev[p, t] = b2[t, p]
    b2f = np.ascontiguousarray(b2.T).astype(np.float32)

    if variant == "dmat":
        in_maps = [
            _prep_core_inputs_dmat(nodes, w1b, b1f, w2b, b2f, idx, core)
            for core in range(N_CORES)
        ]
    else:
        in_maps = [
            _prep_core_inputs(nodes, w1b, b1f, w2b, b2f, idx, core,
                              chunk=cfg.get("chunk", CHUNK))
            for core in range(N_CORES)
        ]

    trace = bool(int(os.environ.get("KERNEL_TRACE", "0") or "0"))
    res = run_bass_kernel_spmd(nc, in_maps, list(range(N_CORES)), trace=trace)
    _LAST_RESULTS["res"] = res

    # unshard: per-core out is [128, T*m_per_core] bf16, column t*m + j
    # holding the output row for item (t, core, j)
    outs = np.stack([np.asarray(res.results[c]["out"])
                     for c in range(N_CORES)])        # [c, 128, T*m]
    outs = outs.astype(np.float32).reshape(N_CORES, P, T, M_PER_CORE)
    out_full = outs.transpose(2, 0, 3, 1).reshape(T * MT, E)

    new_nodes = nodes.copy()
    new_nodes[out_idx_np.reshape(-1)] = out_full
    return new_nodes
